# revision 1
# baseline (speedup 1.0000x reference)
"""Trainium2 Bass kernel for single-head (H=1) masked self-attention over
128 independent graphs of 512 nodes (d_model = 512).

Math (per graph b, X = data rows of b, all [512, 512]):
    S  = (1/sqrt(512)) * X Wq^T Wk X^T        (key-mask -> -inf cols)
    A  = softmax(S, axis=-1), masked cols zeroed
    out = A V Wo^T + b,  V = X Wv^T

Device strategy (data-parallel over batch, 16 graphs per NeuronCore):
  * Host folds Wq/Wk into one matrix Wqk = norm * Wq^T @ Wk (512x512), so
    scores take 2 matmuls instead of 3.
  * Host supplies X^T per graph ([i, g]), so no on-device transposes are
    needed anywhere:
        P1T  = matmul(lhsT=Wqk,  rhs=XT)      # (X Wqk)^T      [i', q]
        ST   = matmul(lhsT=XT,   rhs=P1T)     # S^T            [k,  q]
        Pexp = exp(ST + mask_bias[k])         # ACT, bias -30000 on masked k
        V    = matmul(lhsT=XT,   rhs=WvT)     # X Wv^T         [g,  dv]
        HT   = matmul(lhsT=V,    rhs=Pexp)    # (A_unnorm V)^T [dv, q]
        den  = ones^T @ Pexp                  # softmax denominators [1, q]
        out  = matmul(lhsT=HT,   rhs=WoT) * (1/den)[q] + bias  # [q, e]
    Softmax runs in S^T layout (keys on partitions) so the mask bias is a
    per-partition ACT bias and exp(-30000) = 0 reproduces the exact masked
    zeros of the reference.
  * The [1,512] denominator row is transposed to [128,4] with 4 rank-1
    PE matmuls so normalization is a per-partition scalar multiply.
"""

import math

import numpy as np

N_CORES = 8
B = 128          # graphs
G = 512          # nodes per graph
D = 512          # model dim
BPC = B // N_CORES   # graphs per core
P = 128          # SBUF partitions
NC_ = D // P     # 4 chunks of 128

MASK_BIAS = -30000.0

_CACHE: dict = {}


def _build_nc(mm_fast: bool):
    import concourse.tile as tile
    from concourse import bacc, mybir

    f32 = mybir.dt.float32
    mm_dt = mybir.dt.float32r if mm_fast else mybir.dt.float32

    def mm(ap):
        return ap.bitcast(mm_dt) if mm_fast else ap

    nc = bacc.Bacc("TRN2", target_bir_lowering=False, debug=False,
                   num_devices=N_CORES)

    xt_ap = nc.dram_tensor("xt", [BPC, D, G], f32, kind="ExternalInput").ap()
    wqk_ap = nc.dram_tensor("wqk", [D, D], f32, kind="ExternalInput").ap()
    wvt_ap = nc.dram_tensor("wvt", [D, D], f32, kind="ExternalInput").ap()
    wot_ap = nc.dram_tensor("wot", [D, D], f32, kind="ExternalInput").ap()
    biasb_ap = nc.dram_tensor("biasb", [P, D], f32, kind="ExternalInput").ap()
    maskb_ap = nc.dram_tensor("maskb", [P, BPC, NC_], f32,
                              kind="ExternalInput").ap()
    y_ap = nc.dram_tensor("y", [BPC * G, D], f32, kind="ExternalOutput").ap()

    with tile.TileContext(nc) as tc:
        with (
            tc.tile_pool(name="const", bufs=1) as const_pool,
            tc.tile_pool(name="xt", bufs=2) as xt_pool,
            tc.tile_pool(name="p1t", bufs=2) as p1t_pool,
            tc.tile_pool(name="pexp", bufs=2) as pexp_pool,
            tc.tile_pool(name="v", bufs=2) as v_pool,
            tc.tile_pool(name="h", bufs=2) as h_pool,
            tc.tile_pool(name="outp", bufs=2) as out_pool,
            tc.tile_pool(name="small", bufs=2) as small_pool,
            tc.tile_pool(name="psum", bufs=6, space="PSUM") as psum_pool,
            tc.tile_pool(name="psum_row", bufs=1, space="PSUM") as psrow_pool,
            tc.tile_pool(name="psum_dcol", bufs=1, space="PSUM") as psd_pool,
        ):
            # --- one-time constants ---
            wqk_sb = const_pool.tile([P, NC_, D], f32, tag="wqk")
            nc.sync.dma_start(wqk_sb[:],
                              wqk_ap.rearrange("(po pi) j -> pi po j", pi=P))
            wvt_sb = const_pool.tile([P, NC_, D], f32, tag="wvt")
            nc.sync.dma_start(wvt_sb[:],
                              wvt_ap.rearrange("(po pi) j -> pi po j", pi=P))
            wot_sb = const_pool.tile([P, NC_, D], f32, tag="wot")
            nc.sync.dma_start(wot_sb[:],
                              wot_ap.rearrange("(po pi) j -> pi po j", pi=P))
            biasb_sb = const_pool.tile([P, D], f32, tag="biasb")
            nc.sync.dma_start(biasb_sb[:], biasb_ap[:])
            maskb_sb = const_pool.tile([P, BPC, NC_], f32, tag="maskb")
            nc.sync.dma_start(maskb_sb[:], maskb_ap[:])
            ones_col = const_pool.tile([P, 1], f32, tag="ones_col")
            nc.any.memset(ones_col[:], 1.0)
            one_sc = const_pool.tile([1, 1], f32, tag="one_sc")
            nc.any.memset(one_sc[:], 1.0)

            for b in range(BPC):
                # load X^T for this graph: [128, 4, 512] (i on partitions)
                xt_sb = xt_pool.tile([P, NC_, G], f32, tag="xt")
                nc.sync.dma_start(
                    xt_sb[:],
                    xt_ap[b].rearrange("(po pi) g -> pi po g", pi=P))

                # P1T[i', q] = Wqk^T X^T
                p1t_sb = p1t_pool.tile([P, NC_, G], f32, tag="p1t")
                for mc in range(NC_):
                    ps = psum_pool.tile([P, G], f32, tag="big")
                    for kc in range(NC_):
                        nc.tensor.matmul(
                            ps[:],
                            mm(wqk_sb[:, kc, mc * P:(mc + 1) * P]),
                            mm(xt_sb[:, kc, :]),
                            start=(kc == 0), stop=(kc == NC_ - 1))
                    nc.vector.tensor_copy(p1t_sb[:, mc, :], ps[:])

                # V[g, dv] = X Wv^T
                v_sb = v_pool.tile([P, NC_, D], f32, tag="v")
                for mc in range(NC_):
                    ps = psum_pool.tile([P, D], f32, tag="big")
                    for kc in range(NC_):
                        nc.tensor.matmul(
                            ps[:],
                            mm(xt_sb[:, kc, mc * P:(mc + 1) * P]),
                            mm(wvt_sb[:, kc, :]),
                            start=(kc == 0), stop=(kc == NC_ - 1))
                    nc.vector.tensor_copy(v_sb[:, mc, :], ps[:])

                # ST[k, q] = X P1 (scores transposed), then Pexp = exp(+bias)
                pexp_sb = pexp_pool.tile([P, NC_, G], f32, tag="pexp")
                for mc in range(NC_):
                    ps = psum_pool.tile([P, G], f32, tag="big")
                    for kc in range(NC_):
                        nc.tensor.matmul(
                            ps[:],
                            mm(xt_sb[:, kc, mc * P:(mc + 1) * P]),
                            mm(p1t_sb[:, kc, :]),
                            start=(kc == 0), stop=(kc == NC_ - 1))
                    nc.scalar.activation(
                        pexp_sb[:, mc, :], ps[:],
                        mybir.ActivationFunctionType.Exp,
                        bias=maskb_sb[:, b, mc:mc + 1], scale=1.0)

                # denominators: den[1, q] = ones^T Pexp
                ps_row = psrow_pool.tile([1, G], f32, tag="row")
                for kc in range(NC_):
                    nc.tensor.matmul(
                        ps_row[:],
                        mm(ones_col[:]),
                        mm(pexp_sb[:, kc, :]),
                        start=(kc == 0), stop=(kc == NC_ - 1))
                row_sb = small_pool.tile([1, G], f32, tag="row_sb")
                nc.vector.tensor_copy(row_sb[:], ps_row[:])

                # transpose den row -> [128, 4] (4 rank-1 matmuls)
                ps_d = psd_pool.tile([P, NC_], f32, tag="dcol")
                for qc in range(NC_):
                    nc.tensor.matmul(
                        ps_d[:, qc:qc + 1],
                        mm(row_sb[0:1, qc * P:(qc + 1) * P]),
                        mm(one_sc[:]),
                        start=True, stop=True)
                recip_sb = small_pool.tile([P, NC_], f32, tag="recip")
                nc.vector.reciprocal(recip_sb[:], ps_d[:])

                # HT[dv, q] = (A_unnorm V)^T
                h_sb = h_pool.tile([P, NC_, G], f32, tag="h")
                for mc in range(NC_):
                    ps = psum_pool.tile([P, G], f32, tag="big")
                    for kc in range(NC_):
                        nc.tensor.matmul(
                            ps[:],
                            mm(v_sb[:, kc, mc * P:(mc + 1) * P]),
                            mm(pexp_sb[:, kc, :]),
                            start=(kc == 0), stop=(kc == NC_ - 1))
                    nc.vector.tensor_copy(h_sb[:, mc, :], ps[:])

                # out[q, e] = HT^T WoT, normalized per-q, plus output bias
                out_sb = out_pool.tile([P, NC_, D], f32, tag="out")
                for mc in range(NC_):
                    ps = psum_pool.tile([P, D], f32, tag="big")
                    for kc in range(NC_):
                        nc.tensor.matmul(
                            ps[:],
                            mm(h_sb[:, kc, mc * P:(mc + 1) * P]),
                            mm(wot_sb[:, kc, :]),
                            start=(kc == 0), stop=(kc == NC_ - 1))
                    nc.vector.tensor_scalar_mul(
                        out_sb[:, mc, :], ps[:], recip_sb[:, mc:mc + 1])
                    nc.vector.tensor_add(
                        out_sb[:, mc, :], out_sb[:, mc, :], biasb_sb[:])

                nc.sync.dma_start(
                    y_ap[b * G:(b + 1) * G].rearrange("(po pi) e -> pi po e",
                                                      pi=P),
                    out_sb[:])

    nc.compile()
    return nc


def _get_nc(mm_fast: bool = True):
    key = ("nc", mm_fast)
    if key not in _CACHE:
        _CACHE[key] = _build_nc(mm_fast)
    return _CACHE[key]


def _host_prep(data, W_query, W_key, W_value, W_out_w, W_out_b, mask):
    data = np.asarray(data, dtype=np.float32)
    W_query = np.asarray(W_query, dtype=np.float32)
    W_key = np.asarray(W_key, dtype=np.float32)
    W_value = np.asarray(W_value, dtype=np.float32)
    W_out_w = np.asarray(W_out_w, dtype=np.float32)
    W_out_b = np.asarray(W_out_b, dtype=np.float32)
    mask = np.asarray(mask).astype(bool)

    norm = 1.0 / math.sqrt(D)
    wqk = (norm * (W_query.T.astype(np.float64) @ W_key.astype(np.float64))
           ).astype(np.float32)
    wvt = np.ascontiguousarray(W_value.T)
    wot = np.ascontiguousarray(W_out_w.T)
    biasb = np.broadcast_to(W_out_b, (P, D)).copy()
    # [128, B, 4]: maskb[p, b, kc] = bias for key k = kc*128 + p of graph b
    maskb_full = np.where(mask, np.float32(MASK_BIAS), np.float32(0.0)) \
        .reshape(B, NC_, P).transpose(2, 0, 1).copy()
    # X^T per graph: [B, i, g]
    dataT = data.reshape(B, G, D).transpose(0, 2, 1)

    in_maps = []
    for c in range(N_CORES):
        b0 = c * BPC
        in_maps.append({
            "xt": np.ascontiguousarray(dataT[b0:b0 + BPC]),
            "wqk": wqk,
            "wvt": wvt,
            "wot": wot,
            "biasb": biasb,
            "maskb": np.ascontiguousarray(maskb_full[:, b0:b0 + BPC, :]),
        })
    return in_maps


def kernel(data, W_query, W_key, W_value, W_out_w, W_out_b, mask,
           graph_size):
    from concourse.bass_utils import run_bass_kernel_spmd

    assert int(graph_size) == G
    in_maps = _host_prep(data, W_query, W_key, W_value, W_out_w, W_out_b,
                         mask)
    nc = _get_nc()
    res = run_bass_kernel_spmd(nc, in_maps, list(range(N_CORES)))
    y = np.concatenate([res.results[c]["y"] for c in range(N_CORES)], axis=0)
    return np.ascontiguousarray(y, dtype=np.float32)


# revision 5
# speedup vs baseline: 2.9407x; 2.9407x over previous
"""Trainium2 Bass kernel for single-head (H=1) masked self-attention over
128 independent graphs of 512 nodes (d_model = 512).

Math (per graph b, X = data rows of b, all [512, 512]):
    S  = (1/sqrt(512)) * X Wq^T Wk X^T        (key-mask -> -inf cols)
    A  = softmax(S, axis=-1), masked cols zeroed
    out = A V Wo^T + b,  V = X Wv^T

Device strategy (data-parallel over batch, 16 graphs per NeuronCore):
  * Host folds Wq/Wk into one matrix Wqk = norm * Wq^T @ Wk (512x512), so
    scores take 2 matmuls instead of 3.
  * Host supplies X^T per graph ([i, g]), so no on-device transposes are
    needed anywhere:
        P1T  = matmul(lhsT=Wqk,  rhs=XT)      # (X Wqk)^T      [i', q]
        ST   = matmul(lhsT=XT,   rhs=P1T)     # S^T            [k,  q]
        Pexp = exp(ST + mask_bias[k])         # ACT, bias -30000 on masked k
        V    = matmul(lhsT=XT,   rhs=WvT)     # X Wv^T         [g,  dv]
        HT   = matmul(lhsT=V,    rhs=Pexp)    # (A_unnorm V)^T [dv, q]
        den  = ones^T @ Pexp                  # softmax denominators [1, q]
        out  = matmul(lhsT=HT,   rhs=WoT) * (1/den)[q] + bias  # [q, e]
    Softmax runs in S^T layout (keys on partitions) so the mask bias is a
    per-partition ACT bias and exp(-30000) = 0 reproduces the exact masked
    zeros of the reference.
  * The [1,512] denominator row is transposed to [128,4] with 4 rank-1
    PE matmuls so normalization is a per-partition scalar multiply.
"""

import math

import numpy as np

N_CORES = 8
B = 128          # graphs
G = 512          # nodes per graph
D = 512          # model dim
BPC = B // N_CORES   # graphs per core
P = 128          # SBUF partitions
NC_ = D // P     # 4 chunks of 128

MASK_BIAS = -30000.0

_CACHE: dict = {}


def _build_nc(mm_fast: bool):
    import concourse.tile as tile
    from concourse import bacc, mybir

    f32 = mybir.dt.float32
    # float32r: fp32 bits, single-pass PE mode (1 cyc/row at N>=512 vs 4 for
    # exact fp32). All matmul operands must be produced as float32r, so the
    # tiles (and the DRAM tensors they are DMA'd from) use this dtype.
    mdt = mybir.dt.float32r if mm_fast else mybir.dt.float32

    nc = bacc.Bacc("TRN2", target_bir_lowering=False, debug=False,
                   num_devices=N_CORES)

    xt_ap = nc.dram_tensor("xt", [BPC, D, G], mdt, kind="ExternalInput").ap()
    wqk_ap = nc.dram_tensor("wqk", [D, D], mdt, kind="ExternalInput").ap()
    wvt_ap = nc.dram_tensor("wvt", [D, D], mdt, kind="ExternalInput").ap()
    wot_ap = nc.dram_tensor("wot", [D, D], mdt, kind="ExternalInput").ap()
    biasb_ap = nc.dram_tensor("biasb", [P, D], f32, kind="ExternalInput").ap()
    maskb_ap = nc.dram_tensor("maskb", [P, BPC, NC_], f32,
                              kind="ExternalInput").ap()
    y_ap = nc.dram_tensor("y", [BPC * G, D], f32, kind="ExternalOutput").ap()

    with tile.TileContext(nc) as tc:
        with (
            tc.tile_pool(name="const", bufs=1) as const_pool,
            tc.tile_pool(name="xt", bufs=2) as xt_pool,
            tc.tile_pool(name="p1t", bufs=2) as p1t_pool,
            tc.tile_pool(name="pexp", bufs=2) as pexp_pool,
            tc.tile_pool(name="v", bufs=2) as v_pool,
            tc.tile_pool(name="h", bufs=2) as h_pool,
            tc.tile_pool(name="outp", bufs=2) as out_pool,
            tc.tile_pool(name="small", bufs=2) as small_pool,
            tc.tile_pool(name="psum", bufs=6, space="PSUM") as psum_pool,
            tc.tile_pool(name="psum_row", bufs=1, space="PSUM") as psrow_pool,
            tc.tile_pool(name="psum_dcol", bufs=1, space="PSUM") as psd_pool,
        ):
            # --- one-time constants ---
            wqk_sb = const_pool.tile([P, NC_, D], mdt, tag="wqk")
            nc.sync.dma_start(wqk_sb[:],
                              wqk_ap.rearrange("(po pi) j -> pi po j", pi=P))
            wvt_sb = const_pool.tile([P, NC_, D], mdt, tag="wvt")
            nc.sync.dma_start(wvt_sb[:],
                              wvt_ap.rearrange("(po pi) j -> pi po j", pi=P))
            wot_sb = const_pool.tile([P, NC_, D], mdt, tag="wot")
            nc.sync.dma_start(wot_sb[:],
                              wot_ap.rearrange("(po pi) j -> pi po j", pi=P))
            biasb_sb = const_pool.tile([P, D], f32, tag="biasb")
            nc.sync.dma_start(biasb_sb[:], biasb_ap[:])
            maskb_sb = const_pool.tile([P, BPC, NC_], f32, tag="maskb")
            nc.sync.dma_start(maskb_sb[:], maskb_ap[:])
            # denominator / row-transpose matmuls stay plain fp32 (walrus
            # rejects the exotic K=1 / M=1 shapes in f32r mode)
            ones_col = const_pool.tile([P, 1], f32, tag="ones_col")
            nc.any.memset(ones_col[:], 1.0)
            one_sc = const_pool.tile([1, 1], f32, tag="one_sc")
            nc.any.memset(one_sc[:], 1.0)

            for b in range(BPC):
                # load X^T for this graph: [128, 4, 512] (i on partitions)
                xt_sb = xt_pool.tile([P, NC_, G], mdt, tag="xt")
                nc.sync.dma_start(
                    xt_sb[:],
                    xt_ap[b].rearrange("(po pi) g -> pi po g", pi=P))

                # P1T[i', q] = Wqk^T X^T
                p1t_sb = p1t_pool.tile([P, NC_, G], mdt, tag="p1t")
                for mc in range(NC_):
                    ps = psum_pool.tile([P, G], f32, tag="big")
                    for kc in range(NC_):
                        nc.tensor.matmul(
                            ps[:],
                            wqk_sb[:, kc, mc * P:(mc + 1) * P],
                            xt_sb[:, kc, :],
                            start=(kc == 0), stop=(kc == NC_ - 1))
                    nc.vector.tensor_copy(p1t_sb[:, mc, :], ps[:])

                # V[g, dv] = X Wv^T
                v_sb = v_pool.tile([P, NC_, D], mdt, tag="v")
                for mc in range(NC_):
                    ps = psum_pool.tile([P, D], f32, tag="big")
                    for kc in range(NC_):
                        nc.tensor.matmul(
                            ps[:],
                            xt_sb[:, kc, mc * P:(mc + 1) * P],
                            wvt_sb[:, kc, :],
                            start=(kc == 0), stop=(kc == NC_ - 1))
                    nc.vector.tensor_copy(v_sb[:, mc, :], ps[:])

                # ST[k, q] = X P1 (scores transposed), then Pexp = exp(+bias)
                pexp_sb = pexp_pool.tile([P, NC_, G], mdt, tag="pexp")
                for mc in range(NC_):
                    ps = psum_pool.tile([P, G], f32, tag="big")
                    for kc in range(NC_):
                        nc.tensor.matmul(
                            ps[:],
                            xt_sb[:, kc, mc * P:(mc + 1) * P],
                            p1t_sb[:, kc, :],
                            start=(kc == 0), stop=(kc == NC_ - 1))
                    nc.scalar.activation(
                        pexp_sb[:, mc, :], ps[:],
                        mybir.ActivationFunctionType.Exp,
                        bias=maskb_sb[:, b, mc:mc + 1], scale=1.0)

                # denominators: den[1, q] = ones^T Pexp
                ps_row = psrow_pool.tile([1, G], f32, tag="row")
                for kc in range(NC_):
                    nc.tensor.matmul(
                        ps_row[:],
                        ones_col[:],
                        pexp_sb[:, kc, :].bitcast(f32),
                        start=(kc == 0), stop=(kc == NC_ - 1))
                row_sb = small_pool.tile([1, G], f32, tag="row_sb")
                nc.vector.tensor_copy(row_sb[:], ps_row[:])

                # transpose den row -> [128, 4] (4 rank-1 matmuls)
                ps_d = psd_pool.tile([P, NC_], f32, tag="dcol")
                for qc in range(NC_):
                    nc.tensor.matmul(
                        ps_d[:, qc:qc + 1],
                        row_sb[0:1, qc * P:(qc + 1) * P],
                        one_sc[:],
                        start=True, stop=True)
                recip_sb = small_pool.tile([P, NC_], f32, tag="recip")
                nc.vector.reciprocal(recip_sb[:], ps_d[:])

                # HT[dv, q] = (A_unnorm V)^T
                h_sb = h_pool.tile([P, NC_, G], mdt, tag="h")
                for mc in range(NC_):
                    ps = psum_pool.tile([P, G], f32, tag="big")
                    for kc in range(NC_):
                        nc.tensor.matmul(
                            ps[:],
                            v_sb[:, kc, mc * P:(mc + 1) * P],
                            pexp_sb[:, kc, :],
                            start=(kc == 0), stop=(kc == NC_ - 1))
                    nc.vector.tensor_copy(h_sb[:, mc, :], ps[:])

                # out[q, e] = HT^T WoT, normalized per-q, plus output bias
                out_sb = out_pool.tile([P, NC_, D], f32, tag="out")
                for mc in range(NC_):
                    ps = psum_pool.tile([P, D], f32, tag="big")
                    for kc in range(NC_):
                        nc.tensor.matmul(
                            ps[:],
                            h_sb[:, kc, mc * P:(mc + 1) * P],
                            wot_sb[:, kc, :],
                            start=(kc == 0), stop=(kc == NC_ - 1))
                    nc.vector.tensor_scalar_mul(
                        out_sb[:, mc, :], ps[:], recip_sb[:, mc:mc + 1])
                    nc.vector.tensor_add(
                        out_sb[:, mc, :], out_sb[:, mc, :], biasb_sb[:])

                nc.sync.dma_start(
                    y_ap[b * G:(b + 1) * G].rearrange("(po pi) e -> pi po e",
                                                      pi=P),
                    out_sb[:])

    nc.compile()
    return nc


def _get_nc(mm_fast: bool = True):
    key = ("nc", mm_fast)
    if key not in _CACHE:
        _CACHE[key] = _build_nc(mm_fast)
    return _CACHE[key]


def _host_prep(data, W_query, W_key, W_value, W_out_w, W_out_b, mask):
    data = np.asarray(data, dtype=np.float32)
    W_query = np.asarray(W_query, dtype=np.float32)
    W_key = np.asarray(W_key, dtype=np.float32)
    W_value = np.asarray(W_value, dtype=np.float32)
    W_out_w = np.asarray(W_out_w, dtype=np.float32)
    W_out_b = np.asarray(W_out_b, dtype=np.float32)
    mask = np.asarray(mask).astype(bool)

    norm = 1.0 / math.sqrt(D)
    wqk = (norm * (W_query.T.astype(np.float64) @ W_key.astype(np.float64))
           ).astype(np.float32)
    wvt = np.ascontiguousarray(W_value.T)
    wot = np.ascontiguousarray(W_out_w.T)
    biasb = np.broadcast_to(W_out_b, (P, D)).copy()
    # [128, B, 4]: maskb[p, b, kc] = bias for key k = kc*128 + p of graph b
    maskb_full = np.where(mask, np.float32(MASK_BIAS), np.float32(0.0)) \
        .reshape(B, NC_, P).transpose(2, 0, 1).copy()
    # X^T per graph: [B, i, g]
    dataT = data.reshape(B, G, D).transpose(0, 2, 1)

    in_maps = []
    for c in range(N_CORES):
        b0 = c * BPC
        in_maps.append({
            "xt": np.ascontiguousarray(dataT[b0:b0 + BPC]),
            "wqk": wqk,
            "wvt": wvt,
            "wot": wot,
            "biasb": biasb,
            "maskb": np.ascontiguousarray(maskb_full[:, b0:b0 + BPC, :]),
        })
    return in_maps


def kernel(data, W_query, W_key, W_value, W_out_w, W_out_b, mask,
           graph_size):
    from concourse.bass_utils import run_bass_kernel_spmd

    assert int(graph_size) == G
    in_maps = _host_prep(data, W_query, W_key, W_value, W_out_w, W_out_b,
                         mask)
    nc = _get_nc()
    res = run_bass_kernel_spmd(nc, in_maps, list(range(N_CORES)))
    y = np.concatenate([res.results[c]["y"] for c in range(N_CORES)], axis=0)
    return np.ascontiguousarray(y, dtype=np.float32)


# revision 10
# speedup vs baseline: 3.3129x; 1.1266x over previous
"""Trainium2 Bass kernel for single-head (H=1) masked self-attention over
128 independent graphs of 512 nodes (d_model = 512).

Math (per graph b, X = data rows of b, all [512, 512]):
    S  = (1/sqrt(512)) * X Wq^T Wk X^T        (key-mask -> -inf cols)
    A  = softmax(S, axis=-1), masked cols zeroed
    out = A V Wo^T + b,  V = X Wv^T

Device strategy (data-parallel over batch, 16 graphs per NeuronCore):
  * Host folds Wq/Wk into one matrix Wqk = norm * Wq^T @ Wk (512x512), so
    scores take 2 matmuls instead of 3.
  * Host supplies X^T per graph ([i, g]), so no on-device transposes are
    needed anywhere:
        P1T  = matmul(lhsT=Wqk,  rhs=XT)      # (X Wqk)^T      [i', q]
        ST   = matmul(lhsT=XT,   rhs=P1T)     # S^T            [k,  q]
        Pexp = exp(ST + mask_bias[k])         # ACT, bias -30000 on masked k
        V    = matmul(lhsT=XT,   rhs=WvT)     # X Wv^T         [g,  dv]
        HT   = matmul(lhsT=V,    rhs=Pexp)    # (A_unnorm V)^T [dv, q]
        den  = ones^T @ Pexp                  # softmax denominators [1, q]
        out  = matmul(lhsT=HT,   rhs=WoT) * (1/den)[q] + bias  # [q, e]
    Softmax runs in S^T layout (keys on partitions) so the mask bias is a
    per-partition ACT bias and exp(-30000) = 0 reproduces the exact masked
    zeros of the reference.
  * The [1,512] denominator row is transposed to [128,4] with 4 rank-1
    PE matmuls so normalization is a per-partition scalar multiply.
"""

import math

import numpy as np

N_CORES = 8
B = 128          # graphs
G = 512          # nodes per graph
D = 512          # model dim
BPC = B // N_CORES   # graphs per core
P = 128          # SBUF partitions
NC_ = D // P     # 4 chunks of 128

MASK_BIAS = -30000.0

_CACHE: dict = {}


def _build_nc(mm_fast: bool):
    import concourse.tile as tile
    from concourse import bacc, mybir

    f32 = mybir.dt.float32
    # float32r: fp32 bits, single-pass PE mode (1 cyc/row at N>=512 vs 4 for
    # exact fp32). All matmul operands must be produced as float32r, so the
    # tiles (and the DRAM tensors they are DMA'd from) use this dtype.
    mdt = mybir.dt.float32r if mm_fast else mybir.dt.float32

    nc = bacc.Bacc("TRN2", target_bir_lowering=False, debug=False,
                   num_devices=N_CORES)

    xt_ap = nc.dram_tensor("xt", [BPC, D, G], mdt, kind="ExternalInput").ap()
    wqk_ap = nc.dram_tensor("wqk", [D, D], mdt, kind="ExternalInput").ap()
    wvt_ap = nc.dram_tensor("wvt", [D, D], mdt, kind="ExternalInput").ap()
    wot_ap = nc.dram_tensor("wot", [D, D], mdt, kind="ExternalInput").ap()
    biasb_ap = nc.dram_tensor("biasb", [P, D], f32, kind="ExternalInput").ap()
    maskb_ap = nc.dram_tensor("maskb", [P, BPC, NC_], f32,
                              kind="ExternalInput").ap()
    y_ap = nc.dram_tensor("y", [BPC * G, D], f32, kind="ExternalOutput").ap()

    with tile.TileContext(nc) as tc:
        with (
            tc.tile_pool(name="const", bufs=1) as const_pool,
            tc.tile_pool(name="xt", bufs=2) as xt_pool,
            tc.tile_pool(name="p1t", bufs=2) as p1t_pool,
            tc.tile_pool(name="pexp", bufs=2) as pexp_pool,
            tc.tile_pool(name="v", bufs=2) as v_pool,
            tc.tile_pool(name="h", bufs=2) as h_pool,
            tc.tile_pool(name="outp", bufs=2) as out_pool,
            tc.tile_pool(name="small", bufs=2) as small_pool,
            tc.tile_pool(name="psum", bufs=7, space="PSUM") as psum_pool,
            tc.tile_pool(name="psum_dcol", bufs=1, space="PSUM") as psd_pool,
        ):
            # --- one-time constants ---
            wqk_sb = const_pool.tile([P, NC_, D], mdt, tag="wqk")
            nc.sync.dma_start(wqk_sb[:],
                              wqk_ap.rearrange("(po pi) j -> pi po j", pi=P))
            wvt_sb = const_pool.tile([P, NC_, D], mdt, tag="wvt")
            nc.sync.dma_start(wvt_sb[:],
                              wvt_ap.rearrange("(po pi) j -> pi po j", pi=P))
            wot_sb = const_pool.tile([P, NC_, D], mdt, tag="wot")
            nc.sync.dma_start(wot_sb[:],
                              wot_ap.rearrange("(po pi) j -> pi po j", pi=P))
            biasb_sb = const_pool.tile([P, D], f32, tag="biasb")
            nc.sync.dma_start(biasb_sb[:], biasb_ap[:])
            maskb_sb = const_pool.tile([P, BPC, NC_], f32, tag="maskb")
            nc.sync.dma_start(maskb_sb[:], maskb_ap[:])
            # all-ones lhsT for the denominator reduction (f32r needs full
            # 128-column weights, so M=1 is padded to M=128: every output
            # row of the matmul is the same denominator row)
            ones_mat = const_pool.tile([P, P], mdt, tag="ones_mat")
            if mm_fast:
                # memset can't emit float32r directly; stage fp32 + cast-copy
                ones_f32 = const_pool.tile([P, P], f32, tag="ones_f32")
                nc.any.memset(ones_f32[:], 1.0)
                nc.vector.tensor_copy(ones_mat[:], ones_f32[:])
            else:
                nc.any.memset(ones_mat[:], 1.0)
            # rank-1 row->column transpose matmuls stay plain fp32 (walrus
            # rejects K=1 shapes in f32r mode)
            one_sc = const_pool.tile([1, 1], f32, tag="one_sc")
            nc.any.memset(one_sc[:], 1.0)

            for b in range(BPC):
                # load X^T for this graph: [128, 4, 512] (i on partitions)
                xt_sb = xt_pool.tile([P, NC_, G], mdt, tag="xt")
                nc.sync.dma_start(
                    xt_sb[:],
                    xt_ap[b].rearrange("(po pi) g -> pi po g", pi=P))

                # P1T[i', q] = Wqk^T X^T
                p1t_sb = p1t_pool.tile([P, NC_, G], mdt, tag="p1t")
                for mc in range(NC_):
                    ps = psum_pool.tile([P, G], f32, tag="big")
                    for kc in range(NC_):
                        nc.tensor.matmul(
                            ps[:],
                            wqk_sb[:, kc, mc * P:(mc + 1) * P],
                            xt_sb[:, kc, :],
                            start=(kc == 0), stop=(kc == NC_ - 1))
                    nc.vector.tensor_copy(p1t_sb[:, mc, :], ps[:])

                # V[g, dv] = X Wv^T
                v_sb = v_pool.tile([P, NC_, D], mdt, tag="v")
                for mc in range(NC_):
                    ps = psum_pool.tile([P, D], f32, tag="big")
                    for kc in range(NC_):
                        nc.tensor.matmul(
                            ps[:],
                            xt_sb[:, kc, mc * P:(mc + 1) * P],
                            wvt_sb[:, kc, :],
                            start=(kc == 0), stop=(kc == NC_ - 1))
                    nc.vector.tensor_copy(v_sb[:, mc, :], ps[:])

                # ST[k, q] = X P1 (scores transposed), then Pexp = exp(+bias)
                pexp_sb = pexp_pool.tile([P, NC_, G], mdt, tag="pexp")
                for mc in range(NC_):
                    ps = psum_pool.tile([P, G], f32, tag="big")
                    for kc in range(NC_):
                        nc.tensor.matmul(
                            ps[:],
                            xt_sb[:, kc, mc * P:(mc + 1) * P],
                            p1t_sb[:, kc, :],
                            start=(kc == 0), stop=(kc == NC_ - 1))
                    nc.scalar.activation(
                        pexp_sb[:, mc, :], ps[:],
                        mybir.ActivationFunctionType.Exp,
                        bias=maskb_sb[:, b, mc:mc + 1], scale=1.0)

                # denominators: ones^T Pexp -> [128, 512], every row = den[q]
                ps_row = psum_pool.tile([P, G], f32, tag="big")
                for kc in range(NC_):
                    nc.tensor.matmul(
                        ps_row[:],
                        ones_mat[:],
                        pexp_sb[:, kc, :],
                        start=(kc == 0), stop=(kc == NC_ - 1))
                row_sb = small_pool.tile([1, G], f32, tag="row_sb")
                nc.vector.tensor_copy(row_sb[:], ps_row[0:1, :])

                # transpose den row -> [128, 4] (4 rank-1 matmuls)
                ps_d = psd_pool.tile([P, NC_], f32, tag="dcol")
                for qc in range(NC_):
                    nc.tensor.matmul(
                        ps_d[:, qc:qc + 1],
                        row_sb[0:1, qc * P:(qc + 1) * P],
                        one_sc[:],
                        start=True, stop=True)
                recip_sb = small_pool.tile([P, NC_], f32, tag="recip")
                nc.vector.reciprocal(recip_sb[:], ps_d[:])

                # HT[dv, q] = (A_unnorm V)^T
                h_sb = h_pool.tile([P, NC_, G], mdt, tag="h")
                for mc in range(NC_):
                    ps = psum_pool.tile([P, G], f32, tag="big")
                    for kc in range(NC_):
                        nc.tensor.matmul(
                            ps[:],
                            v_sb[:, kc, mc * P:(mc + 1) * P],
                            pexp_sb[:, kc, :],
                            start=(kc == 0), stop=(kc == NC_ - 1))
                    nc.vector.tensor_copy(h_sb[:, mc, :], ps[:])

                # out[q, e] = HT^T WoT, normalized per-q, plus output bias
                out_sb = out_pool.tile([P, NC_, D], f32, tag="out")
                for mc in range(NC_):
                    ps = psum_pool.tile([P, D], f32, tag="big")
                    for kc in range(NC_):
                        nc.tensor.matmul(
                            ps[:],
                            h_sb[:, kc, mc * P:(mc + 1) * P],
                            wot_sb[:, kc, :],
                            start=(kc == 0), stop=(kc == NC_ - 1))
                    # scale on ACT (Copy is resident in every table set, so
                    # no table switching against the Exp ops), bias on DVE
                    nc.scalar.activation(
                        out_sb[:, mc, :], ps[:],
                        mybir.ActivationFunctionType.Copy,
                        scale=recip_sb[:, mc:mc + 1])
                    nc.vector.tensor_add(
                        out_sb[:, mc, :], out_sb[:, mc, :], biasb_sb[:])

                nc.sync.dma_start(
                    y_ap[b * G:(b + 1) * G].rearrange("(po pi) e -> pi po e",
                                                      pi=P),
                    out_sb[:])

    nc.compile()
    return nc


def _get_nc(mm_fast: bool = True):
    key = ("nc", mm_fast)
    if key not in _CACHE:
        _CACHE[key] = _build_nc(mm_fast)
    return _CACHE[key]


def _host_prep(data, W_query, W_key, W_value, W_out_w, W_out_b, mask):
    data = np.asarray(data, dtype=np.float32)
    W_query = np.asarray(W_query, dtype=np.float32)
    W_key = np.asarray(W_key, dtype=np.float32)
    W_value = np.asarray(W_value, dtype=np.float32)
    W_out_w = np.asarray(W_out_w, dtype=np.float32)
    W_out_b = np.asarray(W_out_b, dtype=np.float32)
    mask = np.asarray(mask).astype(bool)

    norm = 1.0 / math.sqrt(D)
    wqk = (norm * (W_query.T.astype(np.float64) @ W_key.astype(np.float64))
           ).astype(np.float32)
    wvt = np.ascontiguousarray(W_value.T)
    wot = np.ascontiguousarray(W_out_w.T)
    biasb = np.broadcast_to(W_out_b, (P, D)).copy()
    # [128, B, 4]: maskb[p, b, kc] = bias for key k = kc*128 + p of graph b
    maskb_full = np.where(mask, np.float32(MASK_BIAS), np.float32(0.0)) \
        .reshape(B, NC_, P).transpose(2, 0, 1).copy()
    # X^T per graph: [B, i, g]
    dataT = data.reshape(B, G, D).transpose(0, 2, 1)

    in_maps = []
    for c in range(N_CORES):
        b0 = c * BPC
        in_maps.append({
            "xt": np.ascontiguousarray(dataT[b0:b0 + BPC]),
            "wqk": wqk,
            "wvt": wvt,
            "wot": wot,
            "biasb": biasb,
            "maskb": np.ascontiguousarray(maskb_full[:, b0:b0 + BPC, :]),
        })
    return in_maps


def kernel(data, W_query, W_key, W_value, W_out_w, W_out_b, mask,
           graph_size):
    from concourse.bass_utils import run_bass_kernel_spmd

    assert int(graph_size) == G
    in_maps = _host_prep(data, W_query, W_key, W_value, W_out_w, W_out_b,
                         mask)
    nc = _get_nc()
    res = run_bass_kernel_spmd(nc, in_maps, list(range(N_CORES)))
    y = np.concatenate([res.results[c]["y"] for c in range(N_CORES)], axis=0)
    return np.ascontiguousarray(y, dtype=np.float32)
